# revision 5
# baseline (speedup 1.0000x reference)
"""Trainium2 Bass kernel for nn_AggregateClusteredSum (segment_reduce).

Strategy (data-parallel over batch, 8 NeuronCores, no collectives):
  - Each core handles B/8 = 8 batches end to end.
  - Segment sums via onehot matmuls accumulating in PSUM, producing
    activations directly in transposed layout [h, 2K+1] per batch.
  - 6-layer MLP in transposed layout (weights are natural [in,out] = lhsT),
    float32r matmuls (full-rate fp32 path), PReLU as max(x, a*x).
  - The whole post-MLP combination (masked S-sum, subtract, correction
    row-move, masks) is folded into one per-batch [2K+1, K+1] matrix AA
    computed on host from the integer cluster ids; device applies it as a
    single matmul per batch. G_mask is a pure host function of cs_o.

kernel(**inputs) -> (G [B,K+1,g] f32, G_mask [B,K+1] f32), matching reference.
"""

import numpy as np

N_CORES = 8


# ----------------------------------------------------------------------------
# Host-side math: combination matrices + G_mask from integer cluster ids.
# ----------------------------------------------------------------------------
def _host_combination(cs_o, n):
    B = cs_o.shape[0]
    cs = np.asarray(cs_o).copy()
    cs[:, n:] = -1
    K = int(cs.max()) + 1
    Ks = cs.max(axis=1)  # [B]
    R = 2 * K + 1
    ids = np.arange(K)
    counts = (cs[:, :, None] == ids[None, None, :]).sum(axis=1)  # [B, K]
    mk = (counts > 0).astype(np.float32)

    AA = np.zeros((B, K + 1, R), np.float32)
    eye = np.eye(K, dtype=np.float32)
    for b in range(B):
        A0 = np.zeros((K + 1, R), np.float32)
        A0[:K, :K] = mk[b][:, None] * (1.0 - eye)
        A0[:K, K:2 * K] = mk[b][:, None] * eye
        A0[K, :K] = 1.0
        A0[K, 2 * K] = 1.0
        need = (Ks[b] >= 0) and (Ks[b] < K - 1)
        if need:
            A0[Ks[b] + 1, :] = A0[K, :].copy()
            A0[K, :] = 0.0
        colmask = np.concatenate([mk[b], mk[b], [1.0]])
        AA[b] = A0 * colmask[None, :]

    G_mask = np.ones((B, K + 1), np.float32)
    for b in range(B):
        if (Ks[b] >= 0) and (Ks[b] < K - 1):
            G_mask[b, Ks[b] + 2:] = 0.0
    return AA, G_mask, K


# ----------------------------------------------------------------------------
# Device program builder (same SPMD program for every core).
# ----------------------------------------------------------------------------
def _build_nc(cfg):
    import concourse.bacc as bacc
    import concourse.mybir as mybir
    import concourse.tile as tile
    from concourse.masks import make_identity

    F32 = mybir.dt.float32
    F32R = mybir.dt.float32r
    I32 = mybir.dt.int32
    BF16 = mybir.dt.bfloat16

    BPC = cfg["BPC"]          # batches per core
    PC = cfg["PC"]            # point chunks of 128 (n_pad // 128)
    K = cfg["K"]
    R = 2 * K + 1
    H = cfg["H"]              # h_dim (256)
    HC = H // 128
    HID = cfg["HID"]          # 1024
    G = cfg["G"]              # 512
    A_VALS = cfg["A_VALS"]    # [a1..a5] python floats
    ZBIAS = cfg["ZBIAS"]
    NRC = cfg["NRC"]          # row-chunk count (2)
    BPR = BPC // NRC          # batches per row-chunk (4)
    RCW = BPR * R             # row-chunk width (260)
    HS_DT = BF16 if cfg["HS_BF16"] else F32
    ACT_DT = BF16 if cfg["ACT_BF16"] else F32R
    WT_DT = BF16 if cfg["WT_BF16"] else F32R

    LAYER_DIMS = [(H, HID), (HID, HID), (HID, HID), (HID, HID), (HID, HID),
                  (HID, G)]

    nc = bacc.Bacc("TRN2", target_bir_lowering=False, debug=False,
                   num_devices=N_CORES)

    # ---- DRAM tensors (per-core shapes) ----
    hs_pts = nc.dram_tensor("hs_pts", [BPC, 128, PC * H], HS_DT,
                            kind="ExternalInput")
    cs_t = nc.dram_tensor("cs_t", [128, BPC * PC], F32, kind="ExternalInput")
    hn_t = nc.dram_tensor("hn_t", [H, BPC], F32, kind="ExternalInput")
    amat = nc.dram_tensor("amat", [BPC, R, K + 1], F32R, kind="ExternalInput")
    w_dram = []
    for li, (fin, fout) in enumerate(LAYER_DIMS):
        w_dram.append(nc.dram_tensor(f"w{li + 1}", [fin, fout], WT_DT,
                                     kind="ExternalInput"))
    if not ZBIAS:
        # bias columns, packed [128, total_oc]: full bias and a*bias
        TOTC = sum(fo // 128 for _, fo in LAYER_DIMS)
        bias_f = nc.dram_tensor("bias_f", [128, TOTC], F32,
                                kind="ExternalInput")
        bias_q = nc.dram_tensor("bias_q", [128, TOTC], F32,
                                kind="ExternalInput")
    g_out = nc.dram_tensor("g_out", [BPC, K + 1, G], F32,
                           kind="ExternalOutput")

    with tile.TileContext(nc) as tc:
        import contextlib
        with contextlib.ExitStack() as ctx:
            consts = ctx.enter_context(tc.tile_pool(name="consts", bufs=1))
            wpool = ctx.enter_context(tc.tile_pool(name="wpool", bufs=2))
            acts = ctx.enter_context(tc.tile_pool(name="acts", bufs=1))
            hsp = ctx.enter_context(tc.tile_pool(name="hsp", bufs=2))
            ohp = ctx.enter_context(tc.tile_pool(name="ohp", bufs=4))
            scr = ctx.enter_context(tc.tile_pool(name="scr", bufs=4))
            gnat = ctx.enter_context(tc.tile_pool(name="gnat", bufs=2))
            gsb = ctx.enter_context(tc.tile_pool(name="gsb", bufs=2))
            psum_seg = ctx.enter_context(
                tc.tile_pool(name="psum_seg", bufs=3, space="PSUM"))
            psum_mlp = ctx.enter_context(
                tc.tile_pool(name="psum_mlp", bufs=3, space="PSUM"))
            psum_t = ctx.enter_context(
                tc.tile_pool(name="psum_t", bufs=1, space="PSUM"))
            psum_g = ctx.enter_context(
                tc.tile_pool(name="psum_g", bufs=1, space="PSUM"))

            # ---- constants ----
            iota_i = consts.tile([128, K], I32, tag="iota_i", name="iota_i")
            nc.gpsimd.iota(iota_i, pattern=[[1, K]], base=0,
                           channel_multiplier=0)
            iota_f = consts.tile([128, K], F32 if HS_DT == F32 else BF16,
                                 tag="iota_f", name="iota_f")
            nc.vector.tensor_copy(out=iota_f, in_=iota_i)
            ident = consts.tile([128, 128], F32, tag="ident", name="ident")
            make_identity(nc, ident)

            cs_sb = consts.tile([128, BPC * PC], F32, tag="cs_sb", name="cs_sb")
            nc.sync.dma_start(out=cs_sb, in_=cs_t.ap())
            hn_sb = consts.tile([128, HC * BPC], F32, tag="hn_sb", name="hn_sb")
            for hcc in range(HC):
                nc.sync.dma_start(
                    out=hn_sb[:, hcc * BPC:(hcc + 1) * BPC],
                    in_=hn_t.ap()[hcc * 128:(hcc + 1) * 128, :])
            amat_sb = []
            for b in range(BPC):
                t = consts.tile([R, K + 1], F32R, tag=f"amat{b}", name=f"amat{b}")
                nc.sync.dma_start(out=t, in_=amat.ap()[b])
                amat_sb.append(t)
            if not ZBIAS:
                bias_f_sb = consts.tile([128, TOTC], F32, tag="bias_f", name="bias_f_sb")
                nc.sync.dma_start(out=bias_f_sb, in_=bias_f.ap())
                bias_q_sb = consts.tile([128, TOTC], F32, tag="bias_q", name="bias_q_sb")
                nc.sync.dma_start(out=bias_q_sb, in_=bias_q.ap())
                oc_base = np.cumsum([0] + [fo // 128 for _, fo in LAYER_DIMS])

            # ---- activation tiles ----
            xT = [[acts.tile([128, RCW], ACT_DT, tag=f"x{ic}_{rc}", name=f"x{ic}_{rc}")
                   for rc in range(NRC)] for ic in range(HC)]
            hA = [[acts.tile([128, RCW], ACT_DT, tag=f"hA{oc}_{rc}", name=f"hA{oc}_{rc}")
                   for rc in range(NRC)] for oc in range(HID // 128)]
            hB = [[acts.tile([128, RCW], ACT_DT, tag=f"hB{oc}_{rc}", name=f"hB{oc}_{rc}")
                   for rc in range(NRC)] for oc in range(HID // 128)]
            gsT = [[acts.tile([128, RCW], F32, tag=f"gs{oc}_{rc}", name=f"gs{oc}_{rc}")
                    for rc in range(NRC)] for oc in range(G // 128)]

            def w_dma(li):
                fin, fout = LAYER_DIMS[li]
                ic_n = fin // 128
                wt = wpool.tile([128, ic_n, fout], WT_DT, tag="W", name=f"w_t{li}")
                nc.sync.dma_start(
                    out=wt,
                    in_=w_dram[li].ap().rearrange("(ic p) f -> p ic f", p=128))
                return wt

            # =======================================================
            # Phase 1: segment sums -> xT   (+ hs DMA stream)
            # =======================================================
            w_tiles = {}
            for b in range(BPC):
                hst = hsp.tile([128, PC, H], HS_DT, tag="hst", name=f"hst{b}")
                nc.sync.dma_start(out=hst, in_=hs_pts.ap()[b].rearrange(
                    "p (c h) -> p c h", c=PC))
                ps = [psum_seg.tile([128, K], mybir.dt.float32,
                                    tag="segp", name=f"segp{b}_{hc}") for hc in range(HC)]
                for c in range(PC):
                    oh = ohp.tile([128, K], F32 if HS_DT == F32 else BF16,
                                  tag="oh", name=f"oh{b}_{c}")
                    nc.vector.tensor_scalar(
                        out=oh, in0=iota_f,
                        scalar1=cs_sb[:, b * PC + c:b * PC + c + 1],
                        scalar2=None, op0=mybir.AluOpType.is_equal)
                    for hc in range(HC):
                        nc.tensor.matmul(
                            ps[hc][:, :],
                            hst[:, c, hc * 128:(hc + 1) * 128],
                            oh[:, :],
                            start=(c == 0), stop=(c == PC - 1))
                rc, col0 = b // BPR, (b % BPR) * R
                for hc in range(HC):
                    hncol = hn_sb[:, hc * BPC + b:hc * BPC + b + 1]
                    nc.vector.tensor_copy(
                        out=xT[hc][rc][:, col0:col0 + K], in_=ps[hc])
                    nc.vector.tensor_scalar(
                        out=xT[hc][rc][:, col0 + K:col0 + 2 * K],
                        in0=ps[hc], scalar1=hncol, scalar2=None,
                        op0=mybir.AluOpType.add)
                    nc.vector.tensor_copy(
                        out=xT[hc][rc][:, col0 + 2 * K:col0 + R], in_=hncol)
                # interleave weight DMAs into the hs stream (FIFO ring order)
                if b == BPR - 1:
                    w_tiles[0] = w_dma(0)
                if b == BPR + 1:
                    w_tiles[1] = w_dma(1)

            # =======================================================
            # Phase 2: MLP (transposed activations)
            # =======================================================
            cur = xT
            for li, (fin, fout) in enumerate(LAYER_DIMS):
                ic_n, oc_n = fin // 128, fout // 128
                if li not in w_tiles:
                    w_tiles[li] = w_dma(li)
                wt = w_tiles[li]
                if li == 5:
                    nxt = gsT
                else:
                    nxt = hA if (li % 2 == 0) else hB
                for rc in range(NRC):
                    for oc in range(oc_n):
                        ps = psum_mlp.tile([128, RCW], mybir.dt.float32,
                                           tag="mlpp", name=f"mlpp{li}_{rc}_{oc}")
                        for ic in range(ic_n):
                            nc.tensor.matmul(
                                ps[:, :],
                                wt[:, ic, oc * 128:(oc + 1) * 128],
                                cur[ic][rc][:, :],
                                start=(ic == 0), stop=(ic == ic_n - 1))
                        if li < 5:
                            a_l = A_VALS[li]
                            t = scr.tile([128, RCW], mybir.dt.float32,
                                         tag="pt", name=f"pt{li}_{rc}_{oc}")
                            if ZBIAS:
                                nc.scalar.mul(out=t, in_=ps, mul=a_l)
                                nc.vector.tensor_max(
                                    out=nxt[oc][rc], in0=ps, in1=t)
                            else:
                                col = int(oc_base[li]) + oc
                                nc.scalar.activation(
                                    out=t, in_=ps,
                                    func=mybir.ActivationFunctionType.Identity,
                                    bias=bias_q_sb[:, col:col + 1],
                                    scale=a_l)
                                u = scr.tile([128, RCW], mybir.dt.float32,
                                             tag="pu", name=f"pu{li}_{rc}_{oc}")
                                nc.vector.tensor_scalar(
                                    out=u, in0=ps,
                                    scalar1=bias_f_sb[:, col:col + 1],
                                    scalar2=None, op0=mybir.AluOpType.add)
                                nc.vector.tensor_max(
                                    out=nxt[oc][rc], in0=u, in1=t)
                        else:
                            if ZBIAS:
                                nc.scalar.copy(out=nxt[oc][rc], in_=ps)
                            else:
                                col = int(oc_base[li]) + oc
                                nc.vector.tensor_scalar(
                                    out=nxt[oc][rc], in0=ps,
                                    scalar1=bias_f_sb[:, col:col + 1],
                                    scalar2=None, op0=mybir.AluOpType.add)
                cur = nxt

            # =======================================================
            # Phase 3: transpose gs, apply AA, DMA out
            # =======================================================
            for b in range(BPC):
                rc, col0 = b // BPR, (b % BPR) * R
                pt = psum_t.tile([R, G], mybir.dt.float32, tag="pt", name=f"ptr{b}")
                for gc in range(G // 128):
                    nc.tensor.transpose(
                        pt[:, gc * 128:(gc + 1) * 128],
                        gsT[gc][rc][:, col0:col0 + R],
                        ident)
                gn = gnat.tile([R, G], F32R, tag="gn", name=f"gn{b}")
                nc.vector.tensor_copy(out=gn, in_=pt)
                pg = psum_g.tile([K + 1, G], mybir.dt.float32, tag="pg", name=f"pg{b}")
                nc.tensor.matmul(pg[:, :], amat_sb[b][:, :], gn[:, :],
                                 start=True, stop=True)
                go = gsb.tile([K + 1, G], F32, tag="go", name=f"go{b}")
                nc.scalar.copy(out=go, in_=pg)
                nc.scalar.dma_start(out=g_out.ap()[b], in_=go)

    nc.compile()
    return nc


_BUILD_CACHE = {}


def _get_nc(cfg):
    key = tuple(sorted((k, v) for k, v in cfg.items()))
    if key not in _BUILD_CACHE:
        _BUILD_CACHE[key] = _build_nc(cfg)
    return _BUILD_CACHE[key]


# ----------------------------------------------------------------------------
# Host entry point.
# ----------------------------------------------------------------------------
def kernel(hs, W1, b1, a1, W2, b2, a2, W3, b3, a3, W4, b4, a4, W5, b5, a5,
           W6, b6, cs_o, n, _run_opts=None):
    from concourse.bass_utils import run_bass_kernel_spmd

    run_opts = _run_opts or {}
    hs = np.asarray(hs)
    cs_o = np.asarray(cs_o)
    n = int(n)
    B, N, H = hs.shape
    Ws = [np.ascontiguousarray(np.asarray(w), dtype=np.float32)
          for w in (W1, W2, W3, W4, W5, W6)]
    bs = [np.asarray(x, dtype=np.float32)
          for x in (b1, b2, b3, b4, b5, b6)]
    a_vals = [float(np.asarray(a).reshape(-1)[0]) for a in (a1, a2, a3, a4, a5)]
    HID, G = Ws[1].shape[0], Ws[5].shape[1]
    assert B % N_CORES == 0
    BPC = B // N_CORES

    AA, G_mask, K = _host_combination(cs_o, n)
    R = 2 * K + 1

    PC = (n + 127) // 128
    n_pad = PC * 128
    zbias = all(np.all(b == 0) for b in bs)

    NRC = 2 if (B // N_CORES) % 2 == 0 else 1
    cfg = dict(BPC=BPC, PC=PC, K=K, H=H, HID=HID, G=G,
               A_VALS=tuple(a_vals), ZBIAS=zbias, NRC=NRC,
               HS_BF16=bool(run_opts.get("hs_bf16", False)),
               ACT_BF16=bool(run_opts.get("act_bf16", False)),
               WT_BF16=bool(run_opts.get("wt_bf16", False)))
    assert (BPC // NRC) * R >= 256 or cfg["ACT_BF16"], \
        "f32r needs moving dim >= 256"

    nc = _get_nc(cfg)

    # ---- host data staging ----
    if n_pad > n:
        hs_use = np.zeros((B, n_pad, H), np.float32)
        hs_use[:, :n, :] = hs[:, :n, :]
    else:
        hs_use = hs[:, :n, :]
    cs_pad = np.full((B, n_pad), -1, np.int32)
    cs_pad[:, :n] = cs_o[:, :n]

    hs_np_dt = np.float32
    if cfg["HS_BF16"]:
        import ml_dtypes
        hs_np_dt = ml_dtypes.bfloat16
    wt_np_dt = np.float32
    if cfg["WT_BF16"]:
        import ml_dtypes
        wt_np_dt = ml_dtypes.bfloat16

    in_maps = []
    for core in range(N_CORES):
        b0 = core * BPC
        sl = slice(b0, b0 + BPC)
        # [BPC, PC, 128, H] -> [BPC, 128, PC*H]  (partition-contiguous rows)
        hsr = (hs_use[sl].reshape(BPC, PC, 128, H).transpose(0, 2, 1, 3)
               .reshape(BPC, 128, PC * H)).astype(hs_np_dt)
        cst = (cs_pad[sl].reshape(BPC, PC, 128).transpose(2, 0, 1)
               .reshape(128, BPC * PC)).astype(np.float32)
        hnt = np.ascontiguousarray(hs[sl, n, :].T, dtype=np.float32)
        m = {
            "hs_pts": np.ascontiguousarray(hsr),
            "cs_t": np.ascontiguousarray(cst),
            "hn_t": hnt,
            "amat": np.ascontiguousarray(AA[sl].transpose(0, 2, 1)),
        }
        for li in range(6):
            m[f"w{li + 1}"] = Ws[li].astype(wt_np_dt)
        if not zbias:
            dims = [w.shape[1] for w in Ws]
            totc = sum(d // 128 for d in dims)
            bf = np.zeros((128, totc), np.float32)
            bq = np.zeros((128, totc), np.float32)
            col = 0
            for li, d in enumerate(dims):
                aa = a_vals[li] if li < 5 else 1.0
                for oc in range(d // 128):
                    bf[:, col] = bs[li][oc * 128:(oc + 1) * 128]
                    bq[:, col] = aa * bf[:, col]
                    col += 1
            m["bias_f"] = bf
            m["bias_q"] = bq
        in_maps.append(m)

    if run_opts.get("sim", False):
        from concourse.bass_interp import CoreSim
        sim_cores = run_opts.get("sim_cores", [0])
        G_full = np.zeros((B, K + 1, G), np.float32)
        for core in sim_cores:
            sim = CoreSim(nc, trace=False)
            for name, arr in in_maps[core].items():
                sim.tensor(name)[:] = arr
            sim.simulate()
            G_full[core * BPC:(core + 1) * BPC] = np.asarray(
                sim.tensor("g_out"))
        if run_opts.get("return_res", False):
            return (G_full, G_mask), None
        return (G_full, G_mask)

    res = run_bass_kernel_spmd(nc, in_maps, core_ids=list(range(N_CORES)),
                               **{k: v for k, v in run_opts.items()
                                  if k in ("trace", "trace_cores", "tmpdir",
                                           "stitch_traces")})

    G_full = np.empty((B, K + 1, G), np.float32)
    for core in range(N_CORES):
        G_full[core * BPC:(core + 1) * BPC] = res.results[core]["g_out"]

    if run_opts.get("return_res", False):
        return (G_full, G_mask), res
    return (G_full, G_mask)


# revision 18
# speedup vs baseline: 1.2602x; 1.2602x over previous
"""Trainium2 Bass kernel for nn_AggregateClusteredSum (segment_reduce).

Strategy (data-parallel over batch, 8 NeuronCores, no collectives):
  - Each core handles B/8 = 8 batches end to end.
  - Segment sums via onehot matmuls accumulating in PSUM, producing
    activations directly in transposed layout [h, 2K+1] per batch.
  - 6-layer MLP in transposed layout (weights are natural [in,out] = lhsT),
    float32r matmuls (full-rate fp32 path), PReLU as max(x, a*x).
  - The whole post-MLP combination (masked S-sum, subtract, correction
    row-move, masks) is folded into one per-batch [2K+1, K+1] matrix AA
    computed on host from the integer cluster ids; device applies it as a
    single matmul per batch. G_mask is a pure host function of cs_o.

kernel(**inputs) -> (G [B,K+1,g] f32, G_mask [B,K+1] f32), matching reference.
"""

import numpy as np

N_CORES = 8


# ----------------------------------------------------------------------------
# Host-side math: combination matrices + G_mask from integer cluster ids.
# ----------------------------------------------------------------------------
def _host_combination(cs_o, n):
    B = cs_o.shape[0]
    cs = np.asarray(cs_o).copy()
    cs[:, n:] = -1
    K = int(cs.max()) + 1
    Ks = cs.max(axis=1)  # [B]
    R = 2 * K + 1
    ids = np.arange(K)
    counts = (cs[:, :, None] == ids[None, None, :]).sum(axis=1)  # [B, K]
    mk = (counts > 0).astype(np.float32)

    AA = np.zeros((B, K + 1, R), np.float32)
    eye = np.eye(K, dtype=np.float32)
    for b in range(B):
        A0 = np.zeros((K + 1, R), np.float32)
        A0[:K, :K] = mk[b][:, None] * (1.0 - eye)
        A0[:K, K:2 * K] = mk[b][:, None] * eye
        A0[K, :K] = 1.0
        A0[K, 2 * K] = 1.0
        need = (Ks[b] >= 0) and (Ks[b] < K - 1)
        if need:
            A0[Ks[b] + 1, :] = A0[K, :].copy()
            A0[K, :] = 0.0
        colmask = np.concatenate([mk[b], mk[b], [1.0]])
        AA[b] = A0 * colmask[None, :]

    G_mask = np.ones((B, K + 1), np.float32)
    for b in range(B):
        if (Ks[b] >= 0) and (Ks[b] < K - 1):
            G_mask[b, Ks[b] + 2:] = 0.0
    return AA, G_mask, K


# ----------------------------------------------------------------------------
# Device program builder (same SPMD program for every core).
# ----------------------------------------------------------------------------
def _build_nc(cfg):
    import concourse.bacc as bacc
    import concourse.mybir as mybir
    import concourse.tile as tile
    from concourse.masks import make_identity

    F32 = mybir.dt.float32
    F32R = mybir.dt.float32r
    I32 = mybir.dt.int32
    BF16 = mybir.dt.bfloat16

    BPC = cfg["BPC"]          # batches per core
    PC = cfg["PC"]            # point chunks of 128 (n_pad // 128)
    K = cfg["K"]
    R = 2 * K + 1
    H = cfg["H"]              # h_dim (256)
    HC = H // 128
    HID = cfg["HID"]          # 1024
    G = cfg["G"]              # 512
    A_VALS = cfg["A_VALS"]    # [a1..a5] python floats
    ZBIAS = cfg["ZBIAS"]
    NRC = cfg["NRC"]          # row-chunk count (2)
    BPR = BPC // NRC          # batches per row-chunk (4)
    RCW = BPR * R             # row-chunk width (260)
    HS_DT = BF16 if cfg["HS_BF16"] else F32R
    ACT_DT = BF16 if cfg["ACT_BF16"] else F32R
    WT_DT = BF16 if cfg["WT_BF16"] else F32R

    LAYER_DIMS = [(H, HID), (HID, HID), (HID, HID), (HID, HID), (HID, HID),
                  (HID, G)]

    nc = bacc.Bacc("TRN2", target_bir_lowering=False, debug=False,
                   num_devices=N_CORES)

    # ---- DRAM tensors (per-core shapes) ----
    hs_pts = nc.dram_tensor("hs_pts", [BPC, 128, PC * H], HS_DT,
                            kind="ExternalInput")
    cs_t = nc.dram_tensor("cs_t", [128, BPC * PC], F32, kind="ExternalInput")
    hn_nat = nc.dram_tensor("hn_nat", [1, BPC * H], HS_DT,
                            kind="ExternalInput")
    cmat = nc.dram_tensor("cmat", [K + 1, R], HS_DT, kind="ExternalInput")
    amat = nc.dram_tensor("amat", [BPC, R, K + 1], F32R, kind="ExternalInput")
    w_dram = []
    for li, (fin, fout) in enumerate(LAYER_DIMS):
        w_dram.append(nc.dram_tensor(f"w{li + 1}", [fin, fout], WT_DT,
                                     kind="ExternalInput"))
    if not ZBIAS:
        # bias columns, packed [128, total_oc]: full bias and a*bias
        TOTC = sum(fo // 128 for _, fo in LAYER_DIMS)
        bias_f = nc.dram_tensor("bias_f", [128, TOTC], F32,
                                kind="ExternalInput")
        bias_q = nc.dram_tensor("bias_q", [128, TOTC], F32,
                                kind="ExternalInput")
    g_out = nc.dram_tensor("g_out", [BPC, K + 1, G], F32,
                           kind="ExternalOutput")

    with tile.TileContext(nc) as tc:
        import contextlib
        with contextlib.ExitStack() as ctx:
            consts = ctx.enter_context(tc.tile_pool(name="consts", bufs=1))
            wpool = ctx.enter_context(tc.tile_pool(name="wpool", bufs=2))
            acts = ctx.enter_context(tc.tile_pool(name="acts", bufs=1))
            hsp = ctx.enter_context(tc.tile_pool(name="hsp", bufs=2))
            ohp = ctx.enter_context(tc.tile_pool(name="ohp", bufs=3))
            hkp = ctx.enter_context(tc.tile_pool(name="hkp", bufs=2))
            hnatp = ctx.enter_context(tc.tile_pool(name="hnatp", bufs=2))
            scr = ctx.enter_context(tc.tile_pool(name="scr", bufs=4))
            gnat = ctx.enter_context(tc.tile_pool(name="gnat", bufs=2))
            gsb = ctx.enter_context(tc.tile_pool(name="gsb", bufs=2))
            psum_seg = ctx.enter_context(
                tc.tile_pool(name="psum_seg", bufs=2, space="PSUM"))
            psum_mlp = ctx.enter_context(
                tc.tile_pool(name="psum_mlp", bufs=3, space="PSUM"))
            psum_misc = ctx.enter_context(
                tc.tile_pool(name="psum_misc", bufs=3, space="PSUM"))

            # ---- constants (cs/cmat/hn first: phase 1 needs them) ----
            cs_sb = consts.tile([128, BPC * PC], F32, tag="cs_sb", name="cs_sb")
            nc.sync.dma_start(out=cs_sb, in_=cs_t.ap())
            cmat_sb = consts.tile([K + 1, R], HS_DT, tag="cmat", name="cmat_sb")
            nc.sync.dma_start(out=cmat_sb, in_=cmat.ap())
            iota_i = consts.tile([128, K], I32, tag="iota_i", name="iota_i")
            nc.gpsimd.iota(iota_i, pattern=[[1, K]], base=0,
                           channel_multiplier=0)
            iota_f = consts.tile([128, K], F32, tag="iota_f", name="iota_f")
            nc.vector.tensor_copy(out=iota_f, in_=iota_i)
            ident = consts.tile([128, 128], F32, tag="ident", name="ident")
            make_identity(nc, ident)
            amat_sb = []
            for b in range(BPC):
                t = consts.tile([R, K + 1], F32R, tag=f"amat{b}", name=f"amat{b}")
                nc.scalar.dma_start(out=t, in_=amat.ap()[b])
                amat_sb.append(t)
            if not ZBIAS:
                bias_f_sb = consts.tile([128, TOTC], F32, tag="bias_f", name="bias_f_sb")
                nc.sync.dma_start(out=bias_f_sb, in_=bias_f.ap())
                bias_q_sb = consts.tile([128, TOTC], F32, tag="bias_q", name="bias_q_sb")
                nc.sync.dma_start(out=bias_q_sb, in_=bias_q.ap())
                oc_base = np.cumsum([0] + [fo // 128 for _, fo in LAYER_DIMS])

            # ---- activation tiles ----
            xT = [[acts.tile([128, RCW], ACT_DT, tag=f"x{ic}_{rc}", name=f"x{ic}_{rc}")
                   for rc in range(NRC)] for ic in range(HC)]
            hA = [[acts.tile([128, RCW], ACT_DT, tag=f"hA{oc}_{rc}", name=f"hA{oc}_{rc}")
                   for rc in range(NRC)] for oc in range(HID // 128)]
            hB = [[acts.tile([128, RCW], ACT_DT, tag=f"hB{oc}_{rc}", name=f"hB{oc}_{rc}")
                   for rc in range(NRC)] for oc in range(HID // 128)]
            gsT = [[acts.tile([128, RCW], F32, tag=f"gs{oc}_{rc}", name=f"gs{oc}_{rc}")
                    for rc in range(NRC)] for oc in range(G // 128)]

            def w_dma(li):
                fin, fout = LAYER_DIMS[li]
                ic_n = fin // 128
                wt = wpool.tile([128, ic_n, fout], WT_DT, tag="W", name=f"w_t{li}")
                nc.sync.dma_start(
                    out=wt,
                    in_=w_dram[li].ap().rearrange("(ic p) f -> p ic f", p=128))
                return wt

            # =======================================================
            # Phase 1: segment sums -> xT   (+ hs DMA stream)
            #   psum_seg[b] [K+1, H] = [onehot.T @ hs_pts ; hn]
            #   H_nat [R, H] = cmat.T @ hk_ext  (folds the +hn rows)
            #   xT = transpose(H_nat)
            # =======================================================
            w_tiles = {}
            for b in range(BPC):
                hst = hsp.tile([128, PC, H], HS_DT, tag="hst", name=f"hst{b}")
                nc.sync.dma_start(out=hst, in_=hs_pts.ap()[b].rearrange(
                    "p (c h) -> p c h", c=PC))
                ps = psum_seg.tile([K, H], mybir.dt.float32,
                                   tag="segp", name=f"segp{b}")
                for c in range(PC):
                    oh = ohp.tile([128, K], HS_DT, tag="oh", name=f"oh{b}_{c}")
                    nc.vector.tensor_scalar(
                        out=oh, in0=iota_f,
                        scalar1=cs_sb[:, b * PC + c:b * PC + c + 1],
                        scalar2=None, op0=mybir.AluOpType.is_equal)
                    nc.tensor.matmul(ps[:, :], oh[:, :], hst[:, c, :],
                                     start=(c == 0), stop=(c == PC - 1))
                hk_ext = hkp.tile([K + 1, H], HS_DT, tag="hk", name=f"hk{b}")
                nc.vector.tensor_copy(out=hk_ext[0:K, :], in_=ps)
                # row K = hn, straight from DRAM (crosses partitions)
                nc.scalar.dma_start(out=hk_ext[K:K + 1, :],
                                    in_=hn_nat.ap()[:, b * H:(b + 1) * H])
                ph = psum_misc.tile([R, H], mybir.dt.float32, tag="misc",
                                    name=f"ph{b}")
                nc.tensor.matmul(ph[:, :], cmat_sb[:, :], hk_ext[:, :],
                                 start=True, stop=True)
                h_nat = hnatp.tile([R, H], F32, tag="hnat", name=f"hnat{b}")
                nc.scalar.copy(out=h_nat, in_=ph)
                rc, col0 = b // BPR, (b % BPR) * R
                for hc in range(HC):
                    px = psum_misc.tile([128, R], mybir.dt.float32,
                                        tag="misc", name=f"px{b}_{hc}")
                    nc.tensor.transpose(
                        px[:, :], h_nat[:, hc * 128:(hc + 1) * 128],
                        ident[0:R, 0:R])
                    nc.vector.tensor_copy(
                        out=xT[hc][rc][:, col0:col0 + R], in_=px)
                # interleave weight DMAs into the hs stream (FIFO ring order)
                if b == BPR - 1:
                    w_tiles[0] = w_dma(0)
                if b == BPR + 1:
                    w_tiles[1] = w_dma(1)

            # =======================================================
            # Phase 2: MLP (transposed activations)
            # =======================================================
            cur = xT
            for li, (fin, fout) in enumerate(LAYER_DIMS):
                ic_n, oc_n = fin // 128, fout // 128
                if li not in w_tiles:
                    w_tiles[li] = w_dma(li)
                wt = w_tiles[li]
                if li == 5:
                    nxt = gsT
                else:
                    nxt = hA if (li % 2 == 0) else hB
                for oc in range(oc_n):
                    pss = [psum_mlp.tile([128, RCW], mybir.dt.float32,
                                         tag="mlpp", name=f"mlpp{li}_{rc}_{oc}")
                           for rc in range(NRC)]
                    for ic in range(ic_n):
                        for rc in range(NRC):
                            nc.tensor.matmul(
                                pss[rc][:, :],
                                wt[:, ic, oc * 128:(oc + 1) * 128],
                                cur[ic][rc][:, :],
                                start=(ic == 0), stop=(ic == ic_n - 1))
                    for rc in range(NRC):
                        ps = pss[rc]
                        if li < 5:
                            a_l = A_VALS[li]
                            t = scr.tile([128, RCW], mybir.dt.float32,
                                         tag="pt", name=f"pt{li}_{rc}_{oc}")
                            if ZBIAS:
                                nc.scalar.mul(out=t, in_=ps, mul=a_l)
                                nc.vector.tensor_max(
                                    out=nxt[oc][rc], in0=ps, in1=t)
                            else:
                                col = int(oc_base[li]) + oc
                                nc.scalar.activation(
                                    out=t, in_=ps,
                                    func=mybir.ActivationFunctionType.Identity,
                                    bias=bias_q_sb[:, col:col + 1],
                                    scale=a_l)
                                u = scr.tile([128, RCW], mybir.dt.float32,
                                             tag="pu", name=f"pu{li}_{rc}_{oc}")
                                nc.vector.tensor_scalar(
                                    out=u, in0=ps,
                                    scalar1=bias_f_sb[:, col:col + 1],
                                    scalar2=None, op0=mybir.AluOpType.add)
                                nc.vector.tensor_max(
                                    out=nxt[oc][rc], in0=u, in1=t)
                        else:
                            if ZBIAS:
                                nc.scalar.copy(out=nxt[oc][rc], in_=ps)
                            else:
                                col = int(oc_base[li]) + oc
                                nc.vector.tensor_scalar(
                                    out=nxt[oc][rc], in0=ps,
                                    scalar1=bias_f_sb[:, col:col + 1],
                                    scalar2=None, op0=mybir.AluOpType.add)
                cur = nxt

            # =======================================================
            # Phase 3: transpose gs, apply AA, DMA out
            # =======================================================
            for b in range(BPC):
                rc, col0 = b // BPR, (b % BPR) * R
                pt = psum_misc.tile([R, G], mybir.dt.float32, tag="misc", name=f"ptr{b}")
                for gc in range(G // 128):
                    nc.tensor.transpose(
                        pt[:, gc * 128:(gc + 1) * 128],
                        gsT[gc][rc][:, col0:col0 + R],
                        ident)
                gn = gnat.tile([R, G], F32R, tag="gn", name=f"gn{b}")
                nc.vector.tensor_copy(out=gn, in_=pt)
                pg = psum_misc.tile([K + 1, G], mybir.dt.float32, tag="misc", name=f"pg{b}")
                nc.tensor.matmul(pg[:, :], amat_sb[b][:, :], gn[:, :],
                                 start=True, stop=True)
                go = gsb.tile([K + 1, G], F32, tag="go", name=f"go{b}")
                nc.scalar.copy(out=go, in_=pg)
                nc.scalar.dma_start(out=g_out.ap()[b], in_=go)

    nc.compile()
    return nc


_BUILD_CACHE = {}


def _get_nc(cfg):
    key = tuple(sorted((k, v) for k, v in cfg.items()))
    if key not in _BUILD_CACHE:
        _BUILD_CACHE[key] = _build_nc(cfg)
    return _BUILD_CACHE[key]


# ----------------------------------------------------------------------------
# Host entry point.
# ----------------------------------------------------------------------------
def kernel(hs, W1, b1, a1, W2, b2, a2, W3, b3, a3, W4, b4, a4, W5, b5, a5,
           W6, b6, cs_o, n, _run_opts=None):
    from concourse.bass_utils import run_bass_kernel_spmd

    run_opts = _run_opts or {}
    hs = np.asarray(hs)
    cs_o = np.asarray(cs_o)
    n = int(n)
    B, N, H = hs.shape
    Ws = [np.ascontiguousarray(np.asarray(w), dtype=np.float32)
          for w in (W1, W2, W3, W4, W5, W6)]
    bs = [np.asarray(x, dtype=np.float32)
          for x in (b1, b2, b3, b4, b5, b6)]
    a_vals = [float(np.asarray(a).reshape(-1)[0]) for a in (a1, a2, a3, a4, a5)]
    HID, G = Ws[1].shape[0], Ws[5].shape[1]
    assert B % N_CORES == 0
    BPC = B // N_CORES

    AA, G_mask, K = _host_combination(cs_o, n)
    R = 2 * K + 1

    PC = (n + 127) // 128
    n_pad = PC * 128
    zbias = all(np.all(b == 0) for b in bs)

    NRC = 2 if (B // N_CORES) % 2 == 0 else 1
    cfg = dict(BPC=BPC, PC=PC, K=K, H=H, HID=HID, G=G,
               A_VALS=tuple(a_vals), ZBIAS=zbias, NRC=NRC,
               HS_BF16=bool(run_opts.get("hs_bf16", False)),
               ACT_BF16=bool(run_opts.get("act_bf16", False)),
               WT_BF16=bool(run_opts.get("wt_bf16", False)))
    assert (BPC // NRC) * R >= 256 or cfg["ACT_BF16"], \
        "f32r needs moving dim >= 256"

    nc = _get_nc(cfg)

    # ---- host data staging ----
    if n_pad > n:
        hs_use = np.zeros((B, n_pad, H), np.float32)
        hs_use[:, :n, :] = hs[:, :n, :]
    else:
        hs_use = hs[:, :n, :]
    cs_pad = np.full((B, n_pad), -1, np.int32)
    cs_pad[:, :n] = cs_o[:, :n]

    hs_np_dt = np.float32
    if cfg["HS_BF16"]:
        import ml_dtypes
        hs_np_dt = ml_dtypes.bfloat16
    wt_np_dt = np.float32
    if cfg["WT_BF16"]:
        import ml_dtypes
        wt_np_dt = ml_dtypes.bfloat16

    # C matrix: H_nat = C @ Hk_ext; device uses lhsT = C.T [K+1, R]
    Cm = np.zeros((R, K + 1), np.float32)
    Cm[:K, :K] = np.eye(K)
    Cm[K:2 * K, :K] = np.eye(K)
    Cm[K:2 * K, K] = 1.0
    Cm[2 * K, K] = 1.0
    cmat_host = np.ascontiguousarray(Cm.T).astype(hs_np_dt)

    in_maps = []
    for core in range(N_CORES):
        b0 = core * BPC
        sl = slice(b0, b0 + BPC)
        # [BPC, PC, 128, H] -> [BPC, 128, PC*H]  (partition-contiguous rows)
        hsr = (hs_use[sl].reshape(BPC, PC, 128, H).transpose(0, 2, 1, 3)
               .reshape(BPC, 128, PC * H)).astype(hs_np_dt)
        cst = (cs_pad[sl].reshape(BPC, PC, 128).transpose(2, 0, 1)
               .reshape(128, BPC * PC)).astype(np.float32)
        m = {
            "hs_pts": np.ascontiguousarray(hsr),
            "cs_t": np.ascontiguousarray(cst),
            "hn_nat": np.ascontiguousarray(
                hs[sl, n, :].reshape(1, BPC * H)).astype(hs_np_dt),
            "cmat": cmat_host,
            "amat": np.ascontiguousarray(AA[sl].transpose(0, 2, 1)),
        }
        for li in range(6):
            m[f"w{li + 1}"] = Ws[li].astype(wt_np_dt)
        if not zbias:
            dims = [w.shape[1] for w in Ws]
            totc = sum(d // 128 for d in dims)
            bf = np.zeros((128, totc), np.float32)
            bq = np.zeros((128, totc), np.float32)
            col = 0
            for li, d in enumerate(dims):
                aa = a_vals[li] if li < 5 else 1.0
                for oc in range(d // 128):
                    bf[:, col] = bs[li][oc * 128:(oc + 1) * 128]
                    bq[:, col] = aa * bf[:, col]
                    col += 1
            m["bias_f"] = bf
            m["bias_q"] = bq
        in_maps.append(m)

    if run_opts.get("sim", False):
        from concourse.bass_interp import CoreSim
        sim_cores = run_opts.get("sim_cores", [0])
        G_full = np.zeros((B, K + 1, G), np.float32)
        for core in sim_cores:
            sim = CoreSim(nc, trace=False)
            for name, arr in in_maps[core].items():
                sim.tensor(name)[:] = arr
            sim.simulate()
            G_full[core * BPC:(core + 1) * BPC] = np.asarray(
                sim.tensor("g_out"))
        if run_opts.get("return_res", False):
            return (G_full, G_mask), None
        return (G_full, G_mask)

    res = run_bass_kernel_spmd(nc, in_maps, core_ids=list(range(N_CORES)),
                               **{k: v for k, v in run_opts.items()
                                  if k in ("trace", "trace_cores", "tmpdir",
                                           "stitch_traces")})

    G_full = np.empty((B, K + 1, G), np.float32)
    for core in range(N_CORES):
        G_full[core * BPC:(core + 1) * BPC] = res.results[core]["g_out"]

    if run_opts.get("return_res", False):
        return (G_full, G_mask), res
    return (G_full, G_mask)


# revision 27
# speedup vs baseline: 1.3355x; 1.0598x over previous
"""Trainium2 Bass kernel for nn_AggregateClusteredSum (segment_reduce).

Strategy (data-parallel over batch, 8 NeuronCores, no collectives):
  - Each core handles B/8 = 8 batches end to end.
  - Segment sums via onehot matmuls accumulating in PSUM, producing
    activations directly in transposed layout [h, 2K+1] per batch.
  - 6-layer MLP in transposed layout (weights are natural [in,out] = lhsT),
    float32r matmuls (full-rate fp32 path), PReLU as max(x, a*x).
  - The whole post-MLP combination (masked S-sum, subtract, correction
    row-move, masks) is folded into one per-batch [2K+1, K+1] matrix AA
    computed on host from the integer cluster ids; device applies it as a
    single matmul per batch. G_mask is a pure host function of cs_o.

kernel(**inputs) -> (G [B,K+1,g] f32, G_mask [B,K+1] f32), matching reference.
"""

import numpy as np

N_CORES = 8


# ----------------------------------------------------------------------------
# Host-side math: combination matrices + G_mask from integer cluster ids.
# ----------------------------------------------------------------------------
def _host_combination(cs_o, n):
    B = cs_o.shape[0]
    cs = np.asarray(cs_o).copy()
    cs[:, n:] = -1
    K = int(cs.max()) + 1
    Ks = cs.max(axis=1)  # [B]
    R = 2 * K + 1
    ids = np.arange(K)
    counts = (cs[:, :, None] == ids[None, None, :]).sum(axis=1)  # [B, K]
    mk = (counts > 0).astype(np.float32)

    AA = np.zeros((B, K + 1, R), np.float32)
    eye = np.eye(K, dtype=np.float32)
    for b in range(B):
        A0 = np.zeros((K + 1, R), np.float32)
        A0[:K, :K] = mk[b][:, None] * (1.0 - eye)
        A0[:K, K:2 * K] = mk[b][:, None] * eye
        A0[K, :K] = 1.0
        A0[K, 2 * K] = 1.0
        need = (Ks[b] >= 0) and (Ks[b] < K - 1)
        if need:
            A0[Ks[b] + 1, :] = A0[K, :].copy()
            A0[K, :] = 0.0
        colmask = np.concatenate([mk[b], mk[b], [1.0]])
        AA[b] = A0 * colmask[None, :]

    G_mask = np.ones((B, K + 1), np.float32)
    for b in range(B):
        if (Ks[b] >= 0) and (Ks[b] < K - 1):
            G_mask[b, Ks[b] + 2:] = 0.0
    return AA, G_mask, K


# ----------------------------------------------------------------------------
# Device program builder (same SPMD program for every core).
# ----------------------------------------------------------------------------
def _build_nc(cfg):
    import concourse.bacc as bacc
    import concourse.bass as bass
    import concourse.mybir as mybir
    import concourse.tile as tile
    from concourse.masks import make_identity

    F32 = mybir.dt.float32
    F32R = mybir.dt.float32r
    I32 = mybir.dt.int32
    BF16 = mybir.dt.bfloat16

    BPC = cfg["BPC"]          # batches per core
    PC = cfg["PC"]            # point chunks of 128 (n_pad // 128)
    K = cfg["K"]
    R = 2 * K + 1
    H = cfg["H"]              # h_dim (256)
    HC = H // 128
    HID = cfg["HID"]          # 1024
    G = cfg["G"]              # 512
    A_VALS = cfg["A_VALS"]    # [a1..a5] python floats
    ZBIAS = cfg["ZBIAS"]
    NRC = cfg["NRC"]          # row-chunk count (2)
    BPR = BPC // NRC          # batches per row-chunk (4)
    RCW = BPR * R             # row-chunk width (260)
    HS_DT = BF16 if cfg["HS_BF16"] else F32R
    ACT_DT = BF16 if cfg["ACT_BF16"] else F32R
    WT_DT = BF16 if cfg["WT_BF16"] else F32R

    LAYER_DIMS = [(H, HID), (HID, HID), (HID, HID), (HID, HID), (HID, HID),
                  (HID, G)]

    nc = bacc.Bacc("TRN2", target_bir_lowering=False, debug=False,
                   num_devices=N_CORES)

    # ---- DRAM tensors (per-core shapes) ----
    hs_pts = nc.dram_tensor("hs_pts", [BPC, 128, PC * H], HS_DT,
                            kind="ExternalInput")
    cs_t = nc.dram_tensor("cs_t", [128, BPC * PC], F32, kind="ExternalInput")
    hn_nat = nc.dram_tensor("hn_nat", [1, BPC * H], F32R,
                            kind="ExternalInput")
    cmat = nc.dram_tensor("cmat", [K + 1, R], F32R, kind="ExternalInput")
    amat = nc.dram_tensor("amat", [BPC, R, K + 1], F32R, kind="ExternalInput")
    w_dram = []
    for li, (fin, fout) in enumerate(LAYER_DIMS):
        w_dram.append(nc.dram_tensor(f"w{li + 1}", [fin, fout], WT_DT,
                                     kind="ExternalInput"))
    if not ZBIAS:
        # bias columns, packed [128, total_oc]: full bias and a*bias
        TOTC = sum(fo // 128 for _, fo in LAYER_DIMS)
        bias_f = nc.dram_tensor("bias_f", [128, TOTC], F32,
                                kind="ExternalInput")
        bias_q = nc.dram_tensor("bias_q", [128, TOTC], F32,
                                kind="ExternalInput")
    g_out = nc.dram_tensor("g_out", [BPC, K + 1, G], F32,
                           kind="ExternalOutput")

    with tile.TileContext(nc) as tc:
        import contextlib
        with contextlib.ExitStack() as ctx:
            consts = ctx.enter_context(tc.tile_pool(name="consts", bufs=1))
            wpool = ctx.enter_context(tc.tile_pool(name="wpool", bufs=2))
            acts = ctx.enter_context(tc.tile_pool(name="acts", bufs=1))
            hsp = ctx.enter_context(tc.tile_pool(name="hsp", bufs=2))
            ohp = ctx.enter_context(tc.tile_pool(name="ohp", bufs=3))
            hkp = ctx.enter_context(tc.tile_pool(name="hkp", bufs=2))
            hnatp = ctx.enter_context(tc.tile_pool(name="hnatp", bufs=2))
            scr = ctx.enter_context(tc.tile_pool(name="scr", bufs=4))
            gnat = ctx.enter_context(tc.tile_pool(name="gnat", bufs=2))
            gsb = ctx.enter_context(tc.tile_pool(name="gsb", bufs=2))
            psum_seg = ctx.enter_context(
                tc.tile_pool(name="psum_seg", bufs=2, space="PSUM"))
            psum_mlp = ctx.enter_context(
                tc.tile_pool(name="psum_mlp", bufs=3, space="PSUM"))
            psum_misc = ctx.enter_context(
                tc.tile_pool(name="psum_misc", bufs=3, space="PSUM"))

            # ---- constants (cs/cmat/hn first: phase 1 needs them) ----
            cs_sb = consts.tile([128, BPC * PC], F32, tag="cs_sb", name="cs_sb")
            nc.sync.dma_start(out=cs_sb, in_=cs_t.ap())
            cmat_sb = consts.tile([K + 1, R], F32R, tag="cmat", name="cmat_sb")
            nc.sync.dma_start(out=cmat_sb, in_=cmat.ap())
            iota_i = consts.tile([128, PC, K], I32, tag="iota_i", name="iota_i")
            nc.gpsimd.iota(iota_i, pattern=[[0, PC], [1, K]], base=0,
                           channel_multiplier=0)
            iota_f = consts.tile([128, PC, K], F32, tag="iota_f", name="iota_f")
            nc.vector.tensor_copy(out=iota_f, in_=iota_i)
            ident = consts.tile([128, 128], F32, tag="ident", name="ident")
            make_identity(nc, ident)
            amat_sb = []
            for b in range(BPC):
                t = consts.tile([R, K + 1], F32R, tag=f"amat{b}", name=f"amat{b}")
                nc.scalar.dma_start(out=t, in_=amat.ap()[b])
                amat_sb.append(t)
            if not ZBIAS:
                bias_f_sb = consts.tile([128, TOTC], F32, tag="bias_f", name="bias_f_sb")
                nc.sync.dma_start(out=bias_f_sb, in_=bias_f.ap())
                bias_q_sb = consts.tile([128, TOTC], F32, tag="bias_q", name="bias_q_sb")
                nc.sync.dma_start(out=bias_q_sb, in_=bias_q.ap())
                oc_base = np.cumsum([0] + [fo // 128 for _, fo in LAYER_DIMS])

            # ---- activation tiles ----
            xT = [[acts.tile([128, RCW], ACT_DT, tag=f"x{ic}_{rc}", name=f"x{ic}_{rc}")
                   for rc in range(NRC)] for ic in range(HC)]
            hA = [[acts.tile([128, RCW], ACT_DT, tag=f"hA{oc}_{rc}", name=f"hA{oc}_{rc}")
                   for rc in range(NRC)] for oc in range(HID // 128)]
            hB = [[acts.tile([128, RCW], ACT_DT, tag=f"hB{oc}_{rc}", name=f"hB{oc}_{rc}")
                   for rc in range(NRC)] for oc in range(HID // 128)]
            gsT = [[acts.tile([128, RCW], F32, tag=f"gs{oc}_{rc}", name=f"gs{oc}_{rc}")
                    for rc in range(NRC)] for oc in range(G // 128)]

            def w_dma(li):
                fin, fout = LAYER_DIMS[li]
                ic_n = fin // 128
                wt = wpool.tile([128, ic_n, fout], WT_DT, tag="W", name=f"w_t{li}")
                nc.sync.dma_start(
                    out=wt,
                    in_=w_dram[li].ap().rearrange("(ic p) f -> p ic f", p=128))
                return wt

            # =======================================================
            # Phase 1: segment sums -> xT   (+ hs DMA stream)
            #   psum_seg[b] [K+1, H] = [onehot.T @ hs_pts ; hn]
            #   H_nat [R, H] = cmat.T @ hk_ext  (folds the +hn rows)
            #   xT = transpose(H_nat)
            # =======================================================
            w_tiles = {}
            for b in range(BPC):
                hst = hsp.tile([128, PC, H], HS_DT, tag="hst", name=f"hst{b}")
                nc.sync.dma_start(out=hst, in_=hs_pts.ap()[b].rearrange(
                    "p (c h) -> p c h", c=PC))
                ps = psum_seg.tile([K, H], mybir.dt.float32,
                                   tag="segp", name=f"segp{b}")
                # one onehot op per batch: out[p,c,k] = (cs[p,c] == k)
                oh = ohp.tile([128, PC, K], HS_DT, tag="oh", name=f"oh{b}")
                cs_slice = cs_sb[:, b * PC:(b + 1) * PC]
                cs_bcast = bass.AP(
                    tensor=cs_slice.tensor, offset=cs_slice.offset,
                    ap=[list(cs_slice.ap[0]), list(cs_slice.ap[1]), [0, K]])
                nc.vector.tensor_tensor(out=oh, in0=cs_bcast, in1=iota_f,
                                        op=mybir.AluOpType.is_equal)
                for c in range(PC):
                    nc.tensor.matmul(ps[:, :], oh[:, c, :], hst[:, c, :],
                                     start=(c == 0), stop=(c == PC - 1))
                hk_ext = hkp.tile([K + 1, H], F32R, tag="hk", name=f"hk{b}")
                nc.vector.tensor_copy(out=hk_ext[0:K, :], in_=ps)
                # row K = hn, straight from DRAM (crosses partitions)
                nc.scalar.dma_start(out=hk_ext[K:K + 1, :],
                                    in_=hn_nat.ap()[:, b * H:(b + 1) * H])
                ph = psum_misc.tile([R, H], mybir.dt.float32, tag="misc",
                                    name=f"ph{b}")
                nc.tensor.matmul(ph[:, :], cmat_sb[:, :], hk_ext[:, :],
                                 start=True, stop=True)
                h_nat = hnatp.tile([R, H], F32, tag="hnat", name=f"hnat{b}")
                nc.scalar.copy(out=h_nat, in_=ph)
                rc, col0 = b // BPR, (b % BPR) * R
                for hc in range(HC):
                    px = psum_misc.tile([128, R], mybir.dt.float32,
                                        tag="misc", name=f"px{b}_{hc}")
                    nc.tensor.transpose(
                        px[:, :], h_nat[:, hc * 128:(hc + 1) * 128],
                        ident[0:R, 0:R])
                    nc.vector.tensor_copy(
                        out=xT[hc][rc][:, col0:col0 + R], in_=px)
                # interleave W1 into the hs stream (FIFO ring order); the
                # rest queue after all hs batches
                if b == BPR - 1:
                    w_tiles[0] = w_dma(0)

            # =======================================================
            # Phase 2: MLP (transposed activations)
            # =======================================================
            cur = xT
            for li, (fin, fout) in enumerate(LAYER_DIMS):
                ic_n, oc_n = fin // 128, fout // 128
                if li not in w_tiles:
                    w_tiles[li] = w_dma(li)
                wt = w_tiles[li]
                if li == 5:
                    nxt = gsT
                else:
                    nxt = hA if (li % 2 == 0) else hB
                for oc in range(oc_n):
                    pss = [psum_mlp.tile([128, RCW], mybir.dt.float32,
                                         tag="mlpp", name=f"mlpp{li}_{rc}_{oc}")
                           for rc in range(NRC)]
                    for ic in range(ic_n):
                        for rc in range(NRC):
                            nc.tensor.matmul(
                                pss[rc][:, :],
                                wt[:, ic, oc * 128:(oc + 1) * 128],
                                cur[ic][rc][:, :],
                                start=(ic == 0), stop=(ic == ic_n - 1))
                    for rc in range(NRC):
                        ps = pss[rc]
                        if li < 5:
                            a_l = A_VALS[li]
                            t = scr.tile([128, RCW], mybir.dt.float32,
                                         tag="pt", name=f"pt{li}_{rc}_{oc}")
                            if ZBIAS:
                                nc.scalar.mul(out=t, in_=ps, mul=a_l)
                                nc.vector.tensor_max(
                                    out=nxt[oc][rc], in0=ps, in1=t)
                            else:
                                col = int(oc_base[li]) + oc
                                nc.scalar.activation(
                                    out=t, in_=ps,
                                    func=mybir.ActivationFunctionType.Identity,
                                    bias=bias_q_sb[:, col:col + 1],
                                    scale=a_l)
                                u = scr.tile([128, RCW], mybir.dt.float32,
                                             tag="pu", name=f"pu{li}_{rc}_{oc}")
                                nc.vector.tensor_scalar(
                                    out=u, in0=ps,
                                    scalar1=bias_f_sb[:, col:col + 1],
                                    scalar2=None, op0=mybir.AluOpType.add)
                                nc.vector.tensor_max(
                                    out=nxt[oc][rc], in0=u, in1=t)
                        else:
                            if ZBIAS:
                                nc.scalar.copy(out=nxt[oc][rc], in_=ps)
                            else:
                                col = int(oc_base[li]) + oc
                                nc.vector.tensor_scalar(
                                    out=nxt[oc][rc], in0=ps,
                                    scalar1=bias_f_sb[:, col:col + 1],
                                    scalar2=None, op0=mybir.AluOpType.add)
                cur = nxt

            # =======================================================
            # Phase 3: transpose gs, apply AA, DMA out
            # =======================================================
            for b in range(BPC):
                rc, col0 = b // BPR, (b % BPR) * R
                pt = psum_misc.tile([R, G], mybir.dt.float32, tag="misc", name=f"ptr{b}")
                for gc in range(G // 128):
                    nc.tensor.transpose(
                        pt[:, gc * 128:(gc + 1) * 128],
                        gsT[gc][rc][:, col0:col0 + R],
                        ident)
                gn = gnat.tile([R, G], F32R, tag="gn", name=f"gn{b}")
                nc.vector.tensor_copy(out=gn, in_=pt)
                pg = psum_misc.tile([K + 1, G], mybir.dt.float32, tag="misc", name=f"pg{b}")
                nc.tensor.matmul(pg[:, :], amat_sb[b][:, :], gn[:, :],
                                 start=True, stop=True)
                go = gsb.tile([K + 1, G], F32, tag="go", name=f"go{b}")
                nc.scalar.copy(out=go, in_=pg)
                nc.scalar.dma_start(out=g_out.ap()[b], in_=go)

    nc.compile()
    return nc


_BUILD_CACHE = {}


def _get_nc(cfg):
    key = tuple(sorted((k, v) for k, v in cfg.items()))
    if key not in _BUILD_CACHE:
        _BUILD_CACHE[key] = _build_nc(cfg)
    return _BUILD_CACHE[key]


# ----------------------------------------------------------------------------
# Host entry point.
# ----------------------------------------------------------------------------
def kernel(hs, W1, b1, a1, W2, b2, a2, W3, b3, a3, W4, b4, a4, W5, b5, a5,
           W6, b6, cs_o, n, _run_opts=None):
    from concourse.bass_utils import run_bass_kernel_spmd

    run_opts = _run_opts or {}
    hs = np.asarray(hs)
    cs_o = np.asarray(cs_o)
    n = int(n)
    B, N, H = hs.shape
    Ws = [np.ascontiguousarray(np.asarray(w), dtype=np.float32)
          for w in (W1, W2, W3, W4, W5, W6)]
    bs = [np.asarray(x, dtype=np.float32)
          for x in (b1, b2, b3, b4, b5, b6)]
    a_vals = [float(np.asarray(a).reshape(-1)[0]) for a in (a1, a2, a3, a4, a5)]
    HID, G = Ws[1].shape[0], Ws[5].shape[1]
    assert B % N_CORES == 0
    BPC = B // N_CORES

    AA, G_mask, K = _host_combination(cs_o, n)
    R = 2 * K + 1

    PC = (n + 127) // 128
    n_pad = PC * 128
    zbias = all(np.all(b == 0) for b in bs)

    NRC = 2 if (B // N_CORES) % 2 == 0 else 1
    cfg = dict(BPC=BPC, PC=PC, K=K, H=H, HID=HID, G=G,
               A_VALS=tuple(a_vals), ZBIAS=zbias, NRC=NRC,
               HS_BF16=bool(run_opts.get("hs_bf16", False)),
               ACT_BF16=bool(run_opts.get("act_bf16", False)),
               WT_BF16=bool(run_opts.get("wt_bf16", False)))
    assert (BPC // NRC) * R >= 256 or cfg["ACT_BF16"], \
        "f32r needs moving dim >= 256"

    nc = _get_nc(cfg)

    # ---- host data staging ----
    if n_pad > n:
        hs_use = np.zeros((B, n_pad, H), np.float32)
        hs_use[:, :n, :] = hs[:, :n, :]
    else:
        hs_use = hs[:, :n, :]
    cs_pad = np.full((B, n_pad), -1, np.int32)
    cs_pad[:, :n] = cs_o[:, :n]

    hs_np_dt = np.float32
    if cfg["HS_BF16"]:
        import ml_dtypes
        hs_np_dt = ml_dtypes.bfloat16
    wt_np_dt = np.float32
    if cfg["WT_BF16"]:
        import ml_dtypes
        wt_np_dt = ml_dtypes.bfloat16

    # C matrix: H_nat = C @ Hk_ext; device uses lhsT = C.T [K+1, R]
    Cm = np.zeros((R, K + 1), np.float32)
    Cm[:K, :K] = np.eye(K)
    Cm[K:2 * K, :K] = np.eye(K)
    Cm[K:2 * K, K] = 1.0
    Cm[2 * K, K] = 1.0
    cmat_host = np.ascontiguousarray(Cm.T).astype(np.float32)

    in_maps = []
    for core in range(N_CORES):
        b0 = core * BPC
        sl = slice(b0, b0 + BPC)
        # [BPC, PC, 128, H] -> [BPC, 128, PC*H]  (partition-contiguous rows)
        hsr = (hs_use[sl].reshape(BPC, PC, 128, H).transpose(0, 2, 1, 3)
               .reshape(BPC, 128, PC * H)).astype(hs_np_dt)
        cst = (cs_pad[sl].reshape(BPC, PC, 128).transpose(2, 0, 1)
               .reshape(128, BPC * PC)).astype(np.float32)
        m = {
            "hs_pts": np.ascontiguousarray(hsr),
            "cs_t": np.ascontiguousarray(cst),
            "hn_nat": np.ascontiguousarray(
                hs[sl, n, :].reshape(1, BPC * H)).astype(np.float32),
            "cmat": cmat_host,
            "amat": np.ascontiguousarray(AA[sl].transpose(0, 2, 1)),
        }
        for li in range(6):
            m[f"w{li + 1}"] = Ws[li].astype(wt_np_dt)
        if not zbias:
            dims = [w.shape[1] for w in Ws]
            totc = sum(d // 128 for d in dims)
            bf = np.zeros((128, totc), np.float32)
            bq = np.zeros((128, totc), np.float32)
            col = 0
            for li, d in enumerate(dims):
                aa = a_vals[li] if li < 5 else 1.0
                for oc in range(d // 128):
                    bf[:, col] = bs[li][oc * 128:(oc + 1) * 128]
                    bq[:, col] = aa * bf[:, col]
                    col += 1
            m["bias_f"] = bf
            m["bias_q"] = bq
        in_maps.append(m)

    if run_opts.get("sim", False):
        from concourse.bass_interp import CoreSim
        sim_cores = run_opts.get("sim_cores", [0])
        G_full = np.zeros((B, K + 1, G), np.float32)
        for core in sim_cores:
            sim = CoreSim(nc, trace=False)
            for name, arr in in_maps[core].items():
                sim.tensor(name)[:] = arr
            sim.simulate()
            G_full[core * BPC:(core + 1) * BPC] = np.asarray(
                sim.tensor("g_out"))
        if run_opts.get("return_res", False):
            return (G_full, G_mask), None
        return (G_full, G_mask)

    res = run_bass_kernel_spmd(nc, in_maps, core_ids=list(range(N_CORES)),
                               **{k: v for k, v in run_opts.items()
                                  if k in ("trace", "trace_cores", "tmpdir",
                                           "stitch_traces")})

    G_full = np.empty((B, K + 1, G), np.float32)
    for core in range(N_CORES):
        G_full[core * BPC:(core + 1) * BPC] = res.results[core]["g_out"]

    if run_opts.get("return_res", False):
        return (G_full, G_mask), res
    return (G_full, G_mask)
